# revision 11
# baseline (speedup 1.0000x reference)
# Trainium2 Bass kernel for nn_ComplementarySystem: two 2-layer conv branches
# (7x7/s2 + 3x3/s2, GAP, fc->2) over x[64,3,224,224], argmax each, spoof/live
# select. Data-parallel over 8 NeuronCores (8 samples each).
#
# fp8(e4m3) DoubleRow version: all conv matmuls contract 2 taps per PE cycle.
#  - conv1: virtual 192-tap space packed as 96 partitions x 2 pair-halves
#    (pair = dx vs dx+2); ONE DR matmul per 4-row chunk (vs 2 bf16 matmuls).
#  - conv2: (kh0,kh2) row-pairs per branch on 64-partition row groups (t on
#    rows 0-63, f on 64-127 issued adjacently -> concurrent), kh1 as normal
#    fp8 passes on the opposite row groups.
#  - h1 stored as H[128, 2(evict-parity), 57, 128(padded pitch)] fp8 so every
#    psum eviction covers all 128 partitions in one instruction.
#
# Self-contained: numpy + ml_dtypes + concourse imports only. No file reads.
import numpy as np
import ml_dtypes

# ---------------- problem constants (hardcoded per spec) ----------------
B = 64
BPC = 8          # samples per core
NCORES = 8
CIN, H, W = 3, 224, 224
C1, C2 = 64, 128
H1, W1 = 112, 112      # conv1 output
H2, W2 = 56, 56        # conv2 output
PL = 116               # padded phase-plane rows/cols
NPOS2 = H2 * W2        # 3136
FLAT = 112 * PL        # one im2col partition's flat run
HP = 128               # padded h1 col pitch (so (kh0,kh2) DR pair stride %16==0)
DYS = [-1, 0, 1, 2]

E4NP = ml_dtypes.float8_e4m3   # IEEE e4m3 (max +-240) == TRN FP8_EXP4

REPS = 1               # repeat the whole sample pipeline (timing aid)
DEBUG_DUMP = False     # add dram dumps of H/G for sample 0 (debugging)
N_ACT_CONV1 = 4        # of the 14 conv1 evictions/sample, how many go to ACT


def _q8(a):
    return np.clip(np.asarray(a, np.float32), -240, 240).astype(E4NP)


# ---------------- host-side layout prep (pure data movement) ----------------
def _phase_planes(x):
    """x [b,3,224,224] f32 -> zero-padded stride-2 phase planes, e4m3
    [b, phy, phx, c, PL, PL]."""
    b = x.shape[0]
    p = np.zeros((b, 2, 2, CIN, PL, PL), dtype=np.float32)
    p[:, 0, 0, :, 1:113, 1:113] = x[:, :, 0::2, 0::2]
    p[:, 0, 1, :, 1:113, 1:113] = x[:, :, 0::2, 1::2]
    p[:, 1, 0, :, 1:113, 1:113] = x[:, :, 1::2, 0::2]
    p[:, 1, 1, :, 1:113, 1:113] = x[:, :, 1::2, 1::2]
    return _q8(p)


def _prep_weights(inp):
    tW1 = _q8(inp["tW1"]).astype(np.float32)   # [64,3,7,7] (quantized values)
    fW1 = _q8(inp["fW1"]).astype(np.float32)
    tW2 = _q8(inp["tW2"]).astype(np.float32)   # [128,64,3,3]
    fW2 = _q8(inp["fW2"]).astype(np.float32)
    tb1, fb1 = np.asarray(inp["tb1"]), np.asarray(inp["fb1"])
    tb2, fb2 = np.asarray(inp["tb2"]), np.asarray(inp["fb2"])
    tWfc, fWfc = np.asarray(inp["tWfc"]), np.asarray(inp["fWfc"])  # [128,2]
    tbfc, fbfc = np.asarray(inp["tbfc"]), np.asarray(inp["fbfc"])  # [2]

    # conv1 DR lhsT [96, 2(pair), 128]; partition p =
    # 48*dxblk + dy_i*12 + phy*6 + phx*3 + c; pair half i covers dx0+2i.
    # kh = 2*dy+2+phy, kw = 2*dx+2+phx; out-of-range taps are zero phantoms.
    # psum cols 0-63 = t, 64-127 = f for BOTH row parities (no swap): branch
    # t lives entirely on partitions 0-63 of H, f on 64-127, so conv2 can
    # keep each psum bank's accumulation group at a single tile_position.
    w1dr = np.zeros((96, 2, 128), np.float32)
    for dxblk, dx0 in enumerate((-1, 0)):
        for pair in (0, 1):
            dx = dx0 + 2 * pair
            for dyi, dy in enumerate(DYS):
                for phy in (0, 1):
                    for phx in (0, 1):
                        for c in range(CIN):
                            p = 48 * dxblk + dyi * 12 + phy * 6 + phx * 3 + c
                            kh = 2 * dy + 2 + phy
                            kw = 2 * dx + 2 + phx
                            if kh <= 6 and kw <= 6:
                                w1dr[p, pair, 0:64] = tW1[:, c, kh, kw]
                                w1dr[p, pair, 64:128] = fW1[:, c, kh, kw]

    b1p = np.zeros((128, 1), dtype=np.float32)
    b1p[0:64, 0], b1p[64:128, 0] = tb1, fb1

    # H layout: H[p<64, e] = t rows of parity e; H[p>=64, e] = f rows.
    # conv2 DR lhsT (kh0,kh2 pairs from e=0): w2A [128, 3(kw), 2(pair), 128]
    # conv2 kh1 lhsT (from e=1), also DR with a zero phantom half (mixing DR
    # and normal matmuls in one psum group is not supported by HW), and the
    # whole group must share one tile_position: w2B [128, 3(kw), 2, 128]
    w2A = np.zeros((128, 3, 2, 128), np.float32)
    w2B = np.zeros((128, 3, 2, 128), np.float32)
    for kw in range(3):
        w2A[0:64, kw, 0, :] = tW2[:, :, 0, kw].T
        w2A[0:64, kw, 1, :] = tW2[:, :, 2, kw].T
        w2A[64:128, kw, 0, :] = fW2[:, :, 0, kw].T
        w2A[64:128, kw, 1, :] = fW2[:, :, 2, kw].T
        w2B[0:64, kw, 0, :] = tW2[:, :, 1, kw].T
        w2B[64:128, kw, 0, :] = fW2[:, :, 1, kw].T

    b2c = np.stack([tb2, fb2], axis=1).astype(np.float32)       # [128,2]
    wfc = np.stack([tWfc, fWfc], axis=1).astype(np.float32)     # [128,2,2]
    bfc = np.stack([tbfc, fbfc], axis=0)[None].astype(np.float32)  # [1,2,2]
    return dict(w1dr=w1dr.astype(E4NP), b1p=b1p,
                w2A=w2A.astype(E4NP), w2B=w2B.astype(E4NP),
                b2c=b2c, wfc=wfc, bfc=bfc)


# ---------------- device program ----------------
def build_nc():
    import concourse.bass as bass
    import concourse.mybir as mybir
    import concourse.tile as tile
    from concourse import bacc
    from contextlib import ExitStack

    f32 = mybir.dt.float32
    f8 = mybir.dt.float8e4
    AF = mybir.ActivationFunctionType
    OP = mybir.AluOpType
    AX = mybir.AxisListType
    DR = mybir.MatmulPerfMode.DoubleRow

    nc = bacc.Bacc(trn_type="TRN2")
    xp_d = nc.dram_tensor("xp", [BPC, 2, 2, CIN, PL, PL], f8, kind="ExternalInput")
    w1dr_d = nc.dram_tensor("w1dr", [96, 2, 128], f8, kind="ExternalInput")
    w2A_d = nc.dram_tensor("w2A", [128, 3, 2, 128], f8, kind="ExternalInput")
    w2B_d = nc.dram_tensor("w2B", [128, 3, 2, 128], f8, kind="ExternalInput")
    b1p_d = nc.dram_tensor("b1p", [128, 1], f32, kind="ExternalInput")
    b2c_d = nc.dram_tensor("b2c", [128, 2], f32, kind="ExternalInput")
    wfc_d = nc.dram_tensor("wfc", [128, 2, 2], f32, kind="ExternalInput")
    bfc_d = nc.dram_tensor("bfc", [1, 2, 2], f32, kind="ExternalInput")
    out_d = nc.dram_tensor("out", [BPC, 2], f32, kind="ExternalOutput")
    marg_d = nc.dram_tensor("marg", [2, BPC], f32, kind="ExternalOutput")
    if DEBUG_DUMP:
        dbg_H_d = nc.dram_tensor("dbg_H", [128, 2, 57, HP], f8,
                                 kind="ExternalOutput")
        dbg_G_d = nc.dram_tensor("dbg_G", [128, 2, BPC], f32,
                                 kind="ExternalOutput")

    # dram element strides of xp [s, phy, phx, c, PL, PL]
    XS_C = PL * PL
    XS_PHX = CIN * XS_C
    XS_PHY = 2 * XS_PHX
    XS_S = 2 * XS_PHY

    with ExitStack() as ctx:
        tc = ctx.enter_context(tile.TileContext(nc))
        wp = ctx.enter_context(tc.tile_pool(name="weights", bufs=1))
        imp = ctx.enter_context(tc.tile_pool(name="im", bufs=2))
        h1p = ctx.enter_context(tc.tile_pool(name="h1", bufs=2))
        scp = ctx.enter_context(tc.tile_pool(name="scratch", bufs=3))
        gp = ctx.enter_context(tc.tile_pool(name="gap", bufs=2))
        pp1 = ctx.enter_context(tc.tile_pool(name="ps1", bufs=2, space="PSUM"))
        pp2 = ctx.enter_context(tc.tile_pool(name="ps2", bufs=1, space="PSUM"))

        # ---- load weights (already in final dtype/layout from host) ----
        w1dr = wp.tile([96, 2, 128], f8, tag="w_w1dr")
        nc.sync.dma_start(w1dr, w1dr_d.ap())
        w2A = wp.tile([128, 3, 2, 128], f8, tag="w_w2A")
        nc.sync.dma_start(w2A, w2A_d.ap())
        w2B = wp.tile([128, 3, 2, 128], f8, tag="w_w2B")
        nc.sync.dma_start(w2B, w2B_d.ap())
        b1p = wp.tile([128, 1], f32, tag="w_b1p")
        nc.sync.dma_start(b1p, b1p_d.ap())
        b2c = wp.tile([128, 2], f32, tag="w_b2c")
        nc.sync.dma_start(b2c, b2c_d.ap())
        wfc = wp.tile([128, 2, 2], f32, tag="w_wfc")
        nc.sync.dma_start(wfc, wfc_d.ap())
        bfc = wp.tile([1, 2, 2], f32, tag="w_bfc")
        nc.sync.dma_start(bfc, bfc_d.ap())

        G = wp.tile([128, 2, BPC], f32)   # GAP sums per (ch, branch, sample)

        for s in [s for _ in range(REPS) for s in range(BPC)]:
            # ---- im2col load: [96, 112, PL] fp8 (dx blocks -1 and 0); the DR
            # pair half 1 (dx+2) is the same data shifted +2 columns, read via
            # the rhs access pattern — no duplicate load needed. ----
            im = imp.tile([96, 112, PL], f8, tag="im")
            for dxblk, dx0 in ((0, -1), (1, 0)):
                src = bass.AP(
                    tensor=xp_d, offset=s * XS_S + (1 + dx0),
                    ap=[[PL, 4], [XS_C, 12], [1, FLAT]])
                eng = nc.sync if dxblk == 0 else nc.gpsimd
                eng.dma_start(out=im[48 * dxblk:48 * dxblk + 48], in_=src)

            H1t = h1p.tile([128, 2, 57, HP], f8, tag="H")
            # zero borders: row 56 both e (kh2 / phantom-pair reach), col 112
            # both e (kw reach)
            nc.vector.memset(H1t[:, :, 56, 0:113], 0.0)
            nc.vector.memset(H1t[:, :, :, 112], 0.0)

            # ---- conv1: 2 parities x 7 groups x 2 chunks; 1 DR matmul per
            # chunk; one full-width eviction per 2-chunk group. ----
            ev_i = 0
            IMP = 112 * PL
            for par in (0, 1):
                for g in range(7):
                    pt = pp1.tile([128, 2, 512], f32, tag="c1")
                    for q in (0, 1):
                        j = 2 * g + q
                        rhs1 = bass.AP(
                            tensor=im.tensor,
                            offset=im.offset + (par + 8 * j) * PL,
                            ap=[[IMP, 96], [2, 2], [2 * PL, 4], [1, 112]])
                        nc.tensor.matmul(
                            pt[:, q, 0:448], w1dr, rhs1,
                            start=True, stop=True, perf_mode=DR)
                    dst = H1t[:, par, 8 * g:8 * g + 8, 0:112]
                    srcp = pt[:, :, 0:448]
                    if ev_i % 7 < (N_ACT_CONV1 + 1) // 2:
                        nc.scalar.activation(out=dst, in_=srcp, func=AF.Relu,
                                             bias=b1p[:, 0:1])
                    else:
                        nc.vector.tensor_scalar(out=dst, in0=srcp,
                                                scalar1=b1p[:, 0:1],
                                                scalar2=0.0, op0=OP.add,
                                                op1=OP.max)
                    ev_i += 1

            # ---- conv2: 4 groups x 2 chunks; DR (kh0,kh2) on one row half,
            # kh1 normal on the other; t/f issued adjacently for row-group
            # concurrency. Weight-reuse: kw outer, chunk inner. ----
            gc = gp.tile([128, 2, 4], f32, tag="gc")
            HPB = 2 * 57 * HP  # partition pitch of H in elements
            e0 = H1t.offset    # free offset of e=0 block
            e1 = H1t.offset + 57 * HP
            for grp in range(4):
                pst = pp2.tile([128, 2, 512], f32, tag="c2t")
                psf = pp2.tile([128, 2, 512], f32, tag="c2f")
                for kw in range(3):
                    for q in (0, 1):
                        oy0 = 7 * (2 * grp + q)
                        # t: kh0+kh2 DR on rows 0-63
                        rhs_t = bass.AP(
                            tensor=H1t.tensor,
                            offset=e0 + oy0 * HP + kw,
                            ap=[[HPB, 64], [HP, 2], [HP, 7], [2, 56]])
                        nc.tensor.matmul(
                            pst[:, q, 0:392], w2A[0:64, kw], rhs_t,
                            start=(kw == 0), stop=False, perf_mode=DR,
                            tile_position=(0, 0), skip_group_check=True)
                        # f: kh0+kh2 DR on rows 64-127
                        rhs_f = bass.AP(
                            tensor=H1t.tensor,
                            offset=e0 + 64 * HPB + oy0 * HP + kw,
                            ap=[[HPB, 64], [HP, 2], [HP, 7], [2, 56]])
                        nc.tensor.matmul(
                            psf[:, q, 0:392], w2A[64:128, kw], rhs_f,
                            start=(kw == 0), stop=False, perf_mode=DR,
                            tile_position=(64, 0), skip_group_check=True)
                for kw in range(3):
                    for q in (0, 1):
                        oy0 = 7 * (2 * grp + q)
                        # t: kh1 (DR with zero phantom half) on rows 0-63
                        rhs_tB = bass.AP(
                            tensor=H1t.tensor,
                            offset=e1 + oy0 * HP + kw,
                            ap=[[HPB, 64], [HP, 2], [HP, 7], [2, 56]])
                        nc.tensor.matmul(
                            pst[:, q, 0:392], w2B[0:64, kw], rhs_tB,
                            start=False, stop=(kw == 2), perf_mode=DR,
                            tile_position=(0, 0), skip_group_check=True)
                        # f: kh1 on rows 64-127
                        rhs_fB = bass.AP(
                            tensor=H1t.tensor,
                            offset=e1 + 64 * HPB + oy0 * HP + kw,
                            ap=[[HPB, 64], [HP, 2], [HP, 7], [2, 56]])
                        nc.tensor.matmul(
                            psf[:, q, 0:392], w2B[64:128, kw], rhs_fB,
                            start=False, stop=(kw == 2), perf_mode=DR,
                            tile_position=(64, 0), skip_group_check=True)
                for br, ps2 in ((0, pst), (1, psf)):
                    scr = scp.tile([128, 2, 512], f8, tag="h2scr")
                    nc.scalar.activation(
                        out=scr[:, :, 0:392], in_=ps2[:, :, 0:392],
                        func=AF.Relu, bias=b2c[:, br:br + 1],
                        accum_out=gc[:, br, grp:grp + 1])
            nc.vector.reduce_sum(out=G[:, 0, s:s + 1], in_=gc[:, 0, :],
                                 axis=AX.X)
            nc.vector.reduce_sum(out=G[:, 1, s:s + 1], in_=gc[:, 1, :],
                                 axis=AX.X)
            if DEBUG_DUMP and s == 0:
                nc.sync.dma_start(out=dbg_H_d.ap(), in_=H1t)

        if DEBUG_DUMP:
            nc.sync.dma_start(out=dbg_G_d.ap(), in_=G)
        # ---- fc + decision tail ----
        wd = scp.tile([128, 2], f32, tag="wd")
        nc.vector.tensor_tensor(out=wd, in0=wfc[:, :, 1], in1=wfc[:, :, 0],
                                op=OP.subtract)
        nc.scalar.mul(out=wd, in_=wd, mul=1.0 / NPOS2)
        bd = scp.tile([1, 2], f32, tag="bd")
        nc.vector.tensor_tensor(out=bd, in0=bfc[0:1, :, 1], in1=bfc[0:1, :, 0],
                                op=OP.subtract)
        psfc = pp1.tile([128, 2, 512], f32, tag="c1")
        nc.tensor.matmul(psfc[0:1, 0, 0:8], wd[:, 0:1], G[:, 0, :],
                         start=True, stop=False, skip_group_check=True)
        nc.tensor.matmul(psfc[0:1, 0, 8:16], wd[:, 1:2], G[:, 1, :],
                         start=False, stop=True, skip_group_check=True)
        d = scp.tile([1, 2, 8], f32, tag="d")
        nc.scalar.activation(out=d[0:1, 0, :], in_=psfc[0:1, 0, 0:8],
                             func=AF.Identity, bias=bd[0:1, 0:1])
        nc.scalar.activation(out=d[0:1, 1, :], in_=psfc[0:1, 0, 8:16],
                             func=AF.Identity, bias=bd[0:1, 1:2])
        nc.sync.dma_start(out=marg_d.ap(), in_=d[0:1].rearrange("p a b -> p (a b)"))
        m = scp.tile([1, 8], f32, tag="m")
        nc.vector.tensor_tensor(out=m, in0=d[0:1, 0, :], in1=d[0:1, 1, :],
                                op=OP.max)
        g = scp.tile([1, 8], f32, tag="g")
        nc.vector.tensor_scalar(out=g, in0=m, scalar1=0.0, scalar2=None,
                                op0=OP.is_gt)
        oi = scp.tile([1, 8, 2], f32, tag="oi")
        nc.vector.tensor_scalar(out=oi[0:1, :, 0], in0=g, scalar1=-20.0,
                                scalar2=10.0, op0=OP.mult, op1=OP.add)
        nc.vector.tensor_scalar(out=oi[0:1, :, 1], in0=g, scalar1=20.0,
                                scalar2=-10.0, op0=OP.mult, op1=OP.add)
        nc.sync.dma_start(out=out_d.ap(), in_=oi[0:1].rearrange("p a b -> p (a b)"))

    nc.compile()
    return nc


_NC_CACHE = {}


def get_nc():
    key = (REPS, DEBUG_DUMP, N_ACT_CONV1)
    if key not in _NC_CACHE:
        _NC_CACHE[key] = build_nc()
    return _NC_CACHE[key]


def make_in_maps(inputs):
    x = np.asarray(inputs["x"], dtype=np.float32)
    planes = _phase_planes(x)                       # fp8 [64,2,2,3,PL,PL]
    wts = _prep_weights(inputs)
    in_maps = []
    for k in range(NCORES):
        m = dict(wts)
        m["xp"] = np.ascontiguousarray(planes[k * BPC:(k + 1) * BPC])
        in_maps.append(m)
    return in_maps


def kernel(**inputs):
    from concourse.bass_utils import run_bass_kernel_spmd
    nc = get_nc()
    in_maps = make_in_maps(inputs)
    res = run_bass_kernel_spmd(nc, in_maps, core_ids=list(range(NCORES)))
    out = np.concatenate([r["out"] for r in res.results], axis=0)
    return out.astype(np.float32)
